# revision 18
# baseline (speedup 1.0000x reference)
"""Trainium2 Bass kernel for 3-layer GAT + global_add_pool + linear head.

Sharding: nodes (and their incoming edges) are partitioned across 8 cores by
dst; the full node-feature table is exchanged per layer with an AllGather;
messages are gathered per-edge with batched indirect DMAs; the segment
softmax + weighted aggregation run as selector-matrix matmuls on the PE with
PSUM accumulation per 128-node block. Partial pooled logits are summed on the
host (the final all-reduce of [64,10] x 8).

Self-contained: no file reads, shapes hardcoded for the problem instance but
builder is parameterized (used by test.py for a small smoke instance).
"""
import math
import numpy as np
from contextlib import ExitStack

import concourse.bass as bass
import concourse.mybir as mybir
import concourse.tile as tile
from concourse.bass import IndirectOffsetOnAxis
from concourse.bass_utils import run_bass_kernel_spmd
from concourse.tile_rust import add_dep_helper
from concourse.masks import make_identity

NCORES = 8
P = 128
H = 4
Ch = 32
HC = 128          # H * Ch
AUG = HC + 2 * H  # 136: [h | alpha_src | alpha_dst]
TBL = HC + H      # 132: gathered row [h | alpha_src]
NEG_SLOPE = 0.2
GRAPHS = 64
OUT = 10
CHUNK_TILES = 32  # tiles (of 128 edges) per indirect-gather chunk
import os as _os0
USE_IDX64 = _os0.environ.get("K_IDX64", "0") == "1"

# instruction types whose BIR struct cannot carry all Tile-emitted waits
_WAIT_CAPS = {
    "InstDMAGatherAnt": 0,
    "InstDMAScatterAddAnt": 0,
    "InstNoOp": 1,
    "InstDrain": 1,
    "InstCollectiveCompute": 1,
}


def _fixup_wait_limits(nc):
    k = 0
    for fn in nc.m.functions:
        for blk in fn.blocks:
            out = []
            for inst in blk.instructions:
                cap = _WAIT_CAPS.get(type(inst).__name__, 1)
                si = inst.sync_info
                if si is not None:
                    waits = list(si.on_wait)
                    if len(waits) > cap:
                        keep, move = waits[:cap], waits[cap:]
                        for w in move:
                            nop = mybir.InstNoOp(name=f"waitfix_{k}", text_hint="wait_fixup")
                            k += 1
                            nop.engine = inst.engine
                            nop.sync_info = type(si)(on_wait=[w], on_update=[])
                            out.append(nop)
                        inst.sync_info = type(si)(on_wait=list(keep), on_update=list(si.on_update))
                out.append(inst)
            blk.instructions = out
    return k


def _prep_edges(src_all, dst_all, per, nb):
    """Per-core edge tiling. Returns uniform tile->block map plus per-core
    index arrays.

    src_all/dst_all: int arrays of all edges (incl. self loops), dst defines
    the owning core. per = nodes per core, nb = node blocks per core.
    """
    core = dst_all // per
    loc = dst_all % per
    blk = loc // P

    # tiles per block, uniform across cores
    tiles_b = np.zeros(nb, np.int64)
    per_core = []
    for c in range(NCORES):
        m = core == c
        s, d, b, dl = src_all[m], dst_all[m], blk[m], (loc[m] % P)
        order = np.argsort(b, kind="stable")
        s, d, b, dl = s[order], d[order], b[order], dl[order]
        cnt = np.bincount(b, minlength=nb)
        tiles_b = np.maximum(tiles_b, (cnt + P - 1) // P)
        per_core.append((s, d, b, dl, cnt))

    tile_base = np.zeros(nb + 1, np.int64)
    tile_base[1:] = np.cumsum(tiles_b)
    T = int(tile_base[-1])
    Tpad = int(math.ceil(T / CHUNK_TILES) * CHUNK_TILES)

    blk_of_tile = np.full(Tpad, nb - 1, np.int64)
    for b in range(nb):
        blk_of_tile[tile_base[b]:tile_base[b + 1]] = b
    # start/stop flags per tile
    start_t = np.zeros(Tpad, bool)
    stop_t = np.zeros(Tpad, bool)
    for b in range(nb):
        w = np.nonzero(blk_of_tile == b)[0]
        start_t[w[0]] = True
        stop_t[w[-1]] = True

    srcs, adids, dlocs = [], [], []
    for c in range(NCORES):
        s, d, b, dl, cnt = per_core[c]
        src_idx = np.zeros((P, Tpad), np.int32)
        ad_idx = np.zeros((P, Tpad), np.int32)
        dloc_f = np.full((P, Tpad), -1.0, np.float32)
        # slot within block
        off = np.zeros(nb + 1, np.int64)
        off[1:] = np.cumsum(cnt)
        slot = np.arange(len(s)) - off[b]
        t = tile_base[b] + slot // P
        p = slot % P
        src_idx[p, t] = s
        ad_idx[p, t] = (d - c * per)
        dloc_f[p, t] = dl
        # int64-entry layout for chunked gathers: dest chunk m of chunk c is
        # (p=m//CT, j=m%CT); its index value goes in the int64 entry m of the
        # chunk's [P, 2*CT] int32 window, walked partition-innermost:
        # int32 position q=2m -> [q%P, q//P].
        CT = CHUNK_TILES
        nch = Tpad // CT
        src64 = np.zeros((P, 2 * Tpad), np.int32)
        ad64 = np.zeros((P, 2 * Tpad), np.int32)
        jj = t - (t // CT) * CT           # tile within chunk
        m = p * CT + jj                    # dest chunk rank (chunk-local)
        # observed HW walk: first entry at int32 pos 0, then int64 entries
        # from byte 4 (low words at odd positions 2m-1)
        q = np.where(m == 0, 0, 2 * m - 1)
        src64[q % P, (t // CT) * (2 * CT) + q // P] = s
        ad64[q % P, (t // CT) * (2 * CT) + q // P] = d
        srcs.append(src64 if USE_IDX64 else src_idx)
        adids.append(ad64 if USE_IDX64 else ad_idx)
        dlocs.append(dloc_f)
    return blk_of_tile, start_t, stop_t, Tpad, srcs, adids, dlocs


def _build(npad, Tpad, blk_of_tile, start_t, stop_t):
    per = npad // NCORES
    nb = per // P
    nlayers = 3
    f32 = mybir.dt.float32

    nc = bass.Bass(num_devices=NCORES)
    # ---- dram I/O
    xT_d = nc.dram_tensor("xT", [P, per], f32, kind="ExternalInput")
    waug_d = nc.dram_tensor("waug", [nlayers, P, AUG], f32, kind="ExternalInput")
    wh_d = nc.dram_tensor("wh", [P, OUT], f32, kind="ExternalInput")
    iota_d = nc.dram_tensor("iota", [P, 32 * P], f32, kind="ExternalInput")
    IW = 2 * Tpad if USE_IDX64 else Tpad
    srcidx_d = nc.dram_tensor("srcidx", [P, IW], mybir.dt.int32, kind="ExternalInput")
    adidx_d = nc.dram_tensor("adidx", [P, Tpad], mybir.dt.int32, kind="ExternalInput")
    dloc_d = nc.dram_tensor("dloc", [P, Tpad], f32, kind="ExternalInput")
    batchf_d = nc.dram_tensor("batchf", [P, nb], f32, kind="ExternalInput")
    out_d = nc.dram_tensor("out", [GRAPHS, OUT], f32, kind="ExternalOutput")
    import os
    dbg = os.environ.get("K_DEBUG") == "1"
    if dbg:
        dbg_h = nc.dram_tensor("dbg_h", [P, nb * TBL], f32, kind="ExternalOutput")
        dbg_hf = nc.dram_tensor("dbg_hf", [P, TBL], f32, kind="ExternalOutput")
        dbg_g = nc.dram_tensor("dbg_g", [P, CHUNK_TILES * TBL], f32, kind="ExternalOutput")
        dbg_x1 = nc.dram_tensor("dbg_x1", [P, per], f32, kind="ExternalOutput")

    h_loc = [nc.dram_tensor(f"h_loc{l}", [per, TBL], f32) for l in range(nlayers)]
    ad_loc = [nc.dram_tensor(f"ad_loc{l}", [per, H], f32) for l in range(nlayers)]
    h_full = [nc.dram_tensor(f"h_full{l}", [npad, TBL], f32, addr_space="Shared")
              for l in range(nlayers)]

    groups = [list(range(NCORES))]
    nchunks = Tpad // CHUNK_TILES

    with ExitStack() as ctx:
        tc = ctx.enter_context(tile.TileContext(nc))
        sb = ctx.enter_context(tc.tile_pool(name="sb", bufs=1))
        sb_g = ctx.enter_context(tc.tile_pool(name="sbg", bufs=2))
        sb_w = ctx.enter_context(tc.tile_pool(name="sbw", bufs=3))
        ps_h = ctx.enter_context(tc.tile_pool(name="psh", bufs=1, space="PSUM"))
        ps_agg = ctx.enter_context(tc.tile_pool(name="psagg", bufs=3, space="PSUM"))
        ps_xp = ctx.enter_context(tc.tile_pool(name="psxp", bufs=1, space="PSUM"))
        ps_fin = ctx.enter_context(tc.tile_pool(name="psfin", bufs=1, space="PSUM"))

        # ---- persistent SBUF state
        xT = sb.tile([P, per], f32)
        nc.sync.dma_start(out=xT[:], in_=xT_d[:])
        waug = sb.tile([P, nlayers, AUG], f32)
        nc.sync.dma_start(out=waug[:],
                          in_=waug_d[:].rearrange("l p a -> p l a"))
        wh = sb.tile([P, OUT], f32)
        nc.sync.dma_start(out=wh[:], in_=wh_d[:])
        iota = sb.tile([P, 32, P], f32)
        nc.sync.dma_start(out=iota[:].rearrange("p a b -> p (a b)"), in_=iota_d[:])
        srci = sb.tile([P, IW], mybir.dt.int32)
        nc.sync.dma_start(out=srci[:], in_=srcidx_d[:])
        adi = sb.tile([P, Tpad], mybir.dt.int32)
        nc.sync.dma_start(out=adi[:], in_=adidx_d[:])
        dloc = sb.tile([P, Tpad, 1], f32)
        nc.sync.dma_start(out=dloc[:].rearrange("p t o -> p (t o)"), in_=dloc_d[:])
        batchf = sb.tile([P, nb, 1], f32)
        nc.sync.dma_start(out=batchf[:].rearrange("p b o -> p (b o)"), in_=batchf_d[:])
        ident = sb.tile([P, P], f32)
        make_identity(nc, ident[:])

        hsb = sb.tile([P, nb, TBL], f32)
        adsb = sb.tile([P, nb, H], f32)
        pooled_ps = ps_fin.tile([GRAPHS, HC], f32)

        for l in range(3):
            # ===== node phase: h_aug = x @ W_aug =====
            for b in range(nb):
                ps = ps_h.tile([P, AUG], f32)
                nc.tensor.matmul(ps[:], lhsT=xT[:, b * P:(b + 1) * P],
                                 rhs=waug[:, l, :], start=True, stop=True)
                nc.vector.tensor_copy(out=hsb[:, b, :], in_=ps[:, :TBL])
                nc.vector.tensor_copy(out=adsb[:, b, :], in_=ps[:, TBL:AUG])
            dh = nc.sync.dma_start(
                out=h_loc[l][:].rearrange("(b p) d -> p b d", p=P),
                in_=hsb[:])
            da = nc.sync.dma_start(
                out=ad_loc[l][:].rearrange("(b p) d -> p b d", p=P),
                in_=adsb[:])
            cch = nc.gpsimd.collective_compute(
                "AllGather", mybir.AluOpType.bypass, replica_groups=groups,
                ins=[h_loc[l][:]], outs=[h_full[l][:]])
            add_dep_helper(cch.ins, dh.ins, sync=True, reason="h write before ag")
            if dbg and l == 0:
                nc.sync.dma_start(out=dbg_h[:], in_=hsb[:].rearrange("p b d -> p (b d)"))
                hfs = sb.tile([P, TBL], f32)
                dhf = nc.sync.dma_start(out=hfs[:], in_=h_full[0][0:P, :].rearrange("(o p) d -> p (o d)", p=P))
                add_dep_helper(dhf.ins, cch.ins, sync=True, reason="dbg after ag")
                nc.sync.dma_start(out=dbg_hf[:], in_=hfs[:])

            # ===== edge phase =====
            agg_of_blk = {}
            for cidx in range(nchunks):
                t0 = cidx * CHUNK_TILES
                g = sb_g.tile([P, CHUNK_TILES, TBL], f32, tag="gath")
                ag = sb_g.tile([P, CHUNK_TILES, H], f32, tag="gad")
                for j in range(CHUNK_TILES):
                    gi = nc.gpsimd.indirect_dma_start(
                        out=g[:, j, :], out_offset=None, in_=h_full[l][:],
                        in_offset=IndirectOffsetOnAxis(
                            ap=srci[:, t0 + j:t0 + j + 1], axis=0))
                    add_dep_helper(gi.ins, cch.ins, sync=True, reason="gather after ag")
                    ai = nc.gpsimd.indirect_dma_start(
                        out=ag[:, j, :], out_offset=None, in_=ad_loc[l][:],
                        in_offset=IndirectOffsetOnAxis(
                            ap=adi[:, t0 + j:t0 + j + 1], axis=0))
                    add_dep_helper(ai.ins, da.ins, sync=True, reason="adg after write")
                # chunk-batched edge math
                selc = sb_w.tile([P, CHUNK_TILES, P], f32, tag="sel", bufs=2)
                nc.vector.tensor_tensor(
                    out=selc[:],
                    in0=dloc[:, t0:t0 + CHUNK_TILES, :].to_broadcast(
                        [P, CHUNK_TILES, P]),
                    in1=iota[:], op=mybir.AluOpType.is_equal)
                lgc = sb_w.tile([P, CHUNK_TILES, H], f32, tag="lg")
                nc.vector.tensor_tensor(
                    out=lgc[:], in0=g[:, :, HC:TBL], in1=ag[:],
                    op=mybir.AluOpType.add)
                lrc = sb_w.tile([P, CHUNK_TILES, H], f32, tag="lr")
                nc.scalar.activation(lrc[:], lgc[:],
                                     mybir.ActivationFunctionType.Lrelu,
                                     alpha=NEG_SLOPE)
                exc = sb_w.tile([P, CHUNK_TILES, H], f32, tag="ex")
                nc.scalar.activation(exc[:], lrc[:],
                                     mybir.ActivationFunctionType.Exp)
                msgc = sb_w.tile([P, CHUNK_TILES, TBL], f32, tag="msg", bufs=2)
                for h in range(H):
                    nc.vector.tensor_tensor(
                        out=msgc[:, :, h * Ch:(h + 1) * Ch],
                        in0=g[:, :, h * Ch:(h + 1) * Ch],
                        in1=exc[:, :, h:h + 1].to_broadcast([P, CHUNK_TILES, Ch]),
                        op=mybir.AluOpType.mult)
                nc.vector.tensor_copy(out=msgc[:, :, HC:TBL], in_=exc[:])
                for j in range(CHUNK_TILES):
                    t = t0 + j
                    b = int(blk_of_tile[t])
                    if start_t[t]:
                        agg_of_blk[b] = ps_agg.tile([P, TBL], f32, tag="agg", name=f"agg{l}_{b}")
                    nc.tensor.matmul(agg_of_blk[b][:], lhsT=selc[:, j, :],
                                     rhs=msgc[:, j, :],
                                     start=bool(start_t[t]),
                                     stop=bool(stop_t[t]))
                    if stop_t[t]:
                        agg = agg_of_blk.pop(b)
                        rec = sb_w.tile([P, H], f32, tag="rec")
                        nc.vector.reciprocal(rec[:], agg[:, HC:TBL])
                        xb = sb_w.tile([P, HC], f32, tag="xb")
                        for h in range(H):
                            nc.vector.tensor_tensor(
                                out=xb[:, h * Ch:(h + 1) * Ch],
                                in0=agg[:, h * Ch:(h + 1) * Ch],
                                in1=rec[:, h:h + 1].to_broadcast([P, Ch]),
                                op=mybir.AluOpType.mult)
                        nc.vector.tensor_scalar_max(xb[:], xb[:], 0.0)
                        if l < 2:
                            xps = ps_xp.tile([P, P], f32, tag="xps")
                            nc.tensor.transpose(xps[:], xb[:], ident[:])
                            nc.vector.tensor_copy(
                                out=xT[:, b * P:(b + 1) * P], in_=xps[:])
                        else:
                            bsel = sb_w.tile([P, GRAPHS], f32, tag="bsel")
                            nc.vector.tensor_tensor(
                                out=bsel[:],
                                in0=batchf[:, b, :].to_broadcast([P, GRAPHS]),
                                in1=iota[:, 0, :GRAPHS],
                                op=mybir.AluOpType.is_equal)
                            nc.tensor.matmul(pooled_ps[:], lhsT=bsel[:],
                                             rhs=xb[:], start=(b == 0),
                                             stop=(b == nb - 1))

        if dbg:
            nc.sync.dma_start(out=dbg_x1[:], in_=xT[:])
        # ===== head =====
        pooled_sb = sb.tile([GRAPHS, HC], f32)
        nc.vector.tensor_copy(out=pooled_sb[:], in_=pooled_ps[:])
        pT_ps = ps_xp.tile([P, GRAPHS], f32, tag="xps")
        nc.tensor.transpose(pT_ps[:], pooled_sb[:], ident[:GRAPHS, :GRAPHS])
        pT_sb = sb.tile([P, GRAPHS], f32)
        nc.vector.tensor_copy(out=pT_sb[:], in_=pT_ps[:])
        log_ps = ps_xp.tile([GRAPHS, OUT], f32, tag="logps")
        nc.tensor.matmul(log_ps[:], lhsT=pT_sb[:], rhs=wh[:], start=True, stop=True)
        log_sb = sb.tile([GRAPHS, OUT], f32)
        nc.vector.tensor_copy(out=log_sb[:], in_=log_ps[:])
        nc.sync.dma_start(out=out_d[:], in_=log_sb[:])

    _fixup_wait_limits(nc)
    return nc


def prepare(x, Ws, a_srcs, a_dsts, biases, Wh, bh, edge_index, batch):
    n = x.shape[0]
    npad = int(math.ceil(n / (NCORES * P)) * NCORES * P)
    per = npad // NCORES
    nb = per // P

    x = np.asarray(x, np.float32)
    Ws = [np.asarray(w, np.float32) for w in Ws]
    a_srcs = [np.asarray(a, np.float32) for a in a_srcs]
    a_dsts = [np.asarray(a, np.float32) for a in a_dsts]
    Wh = np.asarray(Wh, np.float32)
    bh = np.asarray(bh, np.float32)
    edge_index = np.asarray(edge_index)
    batch = np.asarray(batch)
    for b in biases:
        assert np.allclose(np.asarray(b), 0.0), "nonzero GAT biases unsupported"

    # W_aug = [W | W@As | W@Ad]
    waugs = []
    for l in range(3):
        As = np.zeros((HC, H), np.float32)
        Ad = np.zeros((HC, H), np.float32)
        for h in range(H):
            As[h * Ch:(h + 1) * Ch, h] = a_srcs[l][h]
            Ad[h * Ch:(h + 1) * Ch, h] = a_dsts[l][h]
        W = Ws[l]
        waugs.append(np.concatenate([W, W @ As, W @ Ad], axis=1))
    waug = np.stack(waugs, 0)  # [3, 128, AUG]

    # edges + self loops (incl. pad nodes, so every row has >=1 edge)
    src_all = np.concatenate([edge_index[0].astype(np.int64),
                              np.arange(npad, dtype=np.int64)])
    dst_all = np.concatenate([edge_index[1].astype(np.int64),
                              np.arange(npad, dtype=np.int64)])
    blk_of_tile, start_t, stop_t, Tpad, srcs, adids, dlocs = _prep_edges(
        src_all, dst_all, per, nb)

    xpad = np.zeros((npad, HC), np.float32)
    xpad[:n] = x
    iota = np.tile(np.arange(P, dtype=np.float32)[None, :], (P, 32))

    batchf_full = np.full(npad, -1.0, np.float32)
    batchf_full[:n] = batch.astype(np.float32)

    nc = _build(npad, Tpad, blk_of_tile, start_t, stop_t)

    in_maps = []
    for c in range(NCORES):
        sl = slice(c * per, (c + 1) * per)
        in_maps.append({
            "xT": np.ascontiguousarray(xpad[sl].T),
            "waug": waug,
            "wh": Wh,
            "iota": iota,
            "srcidx": srcs[c],
            "adidx": adids[c],
            "dloc": dlocs[c],
            "batchf": np.ascontiguousarray(
                batchf_full[sl].reshape(nb, P).T),
            })
    return nc, in_maps


def run_gat(x, Ws, a_srcs, a_dsts, biases, Wh, bh, edge_index, batch):
    nc, in_maps = prepare(x, Ws, a_srcs, a_dsts, biases, Wh, bh,
                          edge_index, batch)
    res = run_bass_kernel_spmd(nc, in_maps, list(range(NCORES)))
    global LAST_EXEC_NS
    LAST_EXEC_NS = getattr(res, "exec_time_ns", None)
    logits = np.zeros((GRAPHS, OUT), np.float32)
    for c in range(NCORES):
        logits += res.results[c]["out"]
    return logits + bh


def kernel(**inputs):
    return np.asarray(run_gat(
        inputs["x"], inputs["Ws"], inputs["a_srcs"], inputs["a_dsts"],
        inputs["biases"], inputs["Wh"], inputs["bh"], inputs["edge_index"],
        inputs["batch"]), np.float32)


# revision 20
# speedup vs baseline: 1.1426x; 1.1426x over previous
"""Trainium2 Bass kernel for 3-layer GAT + global_add_pool + linear head.

Sharding: nodes (and their incoming edges) are partitioned across 8 cores by
dst; the full node-feature table is exchanged per layer with an AllGather;
messages are gathered per-edge with batched indirect DMAs; the segment
softmax + weighted aggregation run as selector-matrix matmuls on the PE with
PSUM accumulation per 128-node block. Partial pooled logits are summed on the
host (the final all-reduce of [64,10] x 8).

Self-contained: no file reads, shapes hardcoded for the problem instance but
builder is parameterized (used by test.py for a small smoke instance).
"""
import math
import numpy as np
from contextlib import ExitStack

import concourse.bass as bass
import concourse.mybir as mybir
import concourse.tile as tile
from concourse.bass import IndirectOffsetOnAxis
from concourse.bass_utils import run_bass_kernel_spmd
from concourse.tile_rust import add_dep_helper
from concourse.masks import make_identity

NCORES = 8
P = 128
H = 4
Ch = 32
HC = 128          # H * Ch
AUG = HC + 2 * H  # 136: [h | alpha_src | alpha_dst]
TBL = HC + H      # 132: gathered row [h | alpha_src]
NEG_SLOPE = 0.2
GRAPHS = 64
OUT = 10
CHUNK_TILES = 32  # tiles (of 128 edges) per indirect-gather chunk
import os as _os0
USE_IDX64 = _os0.environ.get("K_IDX64", "0") == "1"

# instruction types whose BIR struct cannot carry all Tile-emitted waits
_WAIT_CAPS = {
    "InstDMAGatherAnt": 0,
    "InstDMAScatterAddAnt": 0,
    "InstNoOp": 1,
    "InstDrain": 1,
    "InstCollectiveCompute": 1,
}


def _fixup_wait_limits(nc):
    k = 0
    for fn in nc.m.functions:
        for blk in fn.blocks:
            out = []
            for inst in blk.instructions:
                cap = _WAIT_CAPS.get(type(inst).__name__, 1)
                si = inst.sync_info
                if si is not None:
                    waits = list(si.on_wait)
                    if len(waits) > cap:
                        keep, move = waits[:cap], waits[cap:]
                        for w in move:
                            nop = mybir.InstNoOp(name=f"waitfix_{k}", text_hint="wait_fixup")
                            k += 1
                            nop.engine = inst.engine
                            nop.sync_info = type(si)(on_wait=[w], on_update=[])
                            out.append(nop)
                        inst.sync_info = type(si)(on_wait=list(keep), on_update=list(si.on_update))
                out.append(inst)
            blk.instructions = out
    return k


def _prep_edges(src_all, dst_all, per, nb):
    """Per-core edge tiling. Returns uniform tile->block map plus per-core
    index arrays.

    src_all/dst_all: int arrays of all edges (incl. self loops), dst defines
    the owning core. per = nodes per core, nb = node blocks per core.
    """
    core = dst_all // per
    loc = dst_all % per
    blk = loc // P

    # tiles per block, uniform across cores
    tiles_b = np.zeros(nb, np.int64)
    per_core = []
    for c in range(NCORES):
        m = core == c
        s, d, b, dl = src_all[m], dst_all[m], blk[m], (loc[m] % P)
        order = np.argsort(b, kind="stable")
        s, d, b, dl = s[order], d[order], b[order], dl[order]
        cnt = np.bincount(b, minlength=nb)
        tiles_b = np.maximum(tiles_b, (cnt + P - 1) // P)
        per_core.append((s, d, b, dl, cnt))

    tile_base = np.zeros(nb + 1, np.int64)
    tile_base[1:] = np.cumsum(tiles_b)
    T = int(tile_base[-1])
    Tpad = int(math.ceil(T / CHUNK_TILES) * CHUNK_TILES)

    blk_of_tile = np.full(Tpad, nb - 1, np.int64)
    for b in range(nb):
        blk_of_tile[tile_base[b]:tile_base[b + 1]] = b
    # start/stop flags per tile
    start_t = np.zeros(Tpad, bool)
    stop_t = np.zeros(Tpad, bool)
    for b in range(nb):
        w = np.nonzero(blk_of_tile == b)[0]
        start_t[w[0]] = True
        stop_t[w[-1]] = True

    srcs, adids, dlocs = [], [], []
    for c in range(NCORES):
        s, d, b, dl, cnt = per_core[c]
        src_idx = np.zeros((P, Tpad), np.int32)
        ad_idx = np.zeros((P, Tpad), np.int32)
        dloc_f = np.full((P, Tpad), -1.0, np.float32)
        # slot within block
        off = np.zeros(nb + 1, np.int64)
        off[1:] = np.cumsum(cnt)
        slot = np.arange(len(s)) - off[b]
        t = tile_base[b] + slot // P
        p = slot % P
        src_idx[p, t] = s
        ad_idx[p, t] = d
        dloc_f[p, t] = dl
        # int64-entry layout for chunked gathers: dest chunk m of chunk c is
        # (p=m//CT, j=m%CT); its index value goes in the int64 entry m of the
        # chunk's [P, 2*CT] int32 window, walked partition-innermost:
        # int32 position q=2m -> [q%P, q//P].
        CT = CHUNK_TILES
        nch = Tpad // CT
        src64 = np.zeros((P, 2 * Tpad), np.int32)
        ad64 = np.zeros((P, 2 * Tpad), np.int32)
        jj = t - (t // CT) * CT           # tile within chunk
        m = p * CT + jj                    # dest chunk rank (chunk-local)
        # observed HW walk: first entry at int32 pos 0, then int64 entries
        # from byte 4 (low words at odd positions 2m-1)
        q = np.where(m == 0, 0, 2 * m - 1)
        src64[q % P, (t // CT) * (2 * CT) + q // P] = s
        ad64[q % P, (t // CT) * (2 * CT) + q // P] = d
        srcs.append(src64 if USE_IDX64 else src_idx)
        adids.append(ad64 if USE_IDX64 else ad_idx)
        dlocs.append(dloc_f)
    return blk_of_tile, start_t, stop_t, Tpad, srcs, adids, dlocs


def _build(npad, Tpad, blk_of_tile, start_t, stop_t):
    per = npad // NCORES
    nb = per // P
    nlayers = 3
    f32 = mybir.dt.float32

    nc = bass.Bass(num_devices=NCORES)
    # ---- dram I/O
    xT_d = nc.dram_tensor("xT", [P, per], f32, kind="ExternalInput")
    waug_d = nc.dram_tensor("waug", [nlayers, P, AUG], f32, kind="ExternalInput")
    wh_d = nc.dram_tensor("wh", [P, OUT], f32, kind="ExternalInput")
    iota_d = nc.dram_tensor("iota", [P, 4 * P], f32, kind="ExternalInput")
    IW = 2 * Tpad if USE_IDX64 else Tpad
    srcidx_d = nc.dram_tensor("srcidx", [P, IW], mybir.dt.int32, kind="ExternalInput")
    dloc_d = nc.dram_tensor("dloc", [P, Tpad], f32, kind="ExternalInput")
    batchf_d = nc.dram_tensor("batchf", [P, nb], f32, kind="ExternalInput")
    out_d = nc.dram_tensor("out", [GRAPHS, OUT], f32, kind="ExternalOutput")
    import os
    dbg = os.environ.get("K_DEBUG") == "1"
    if dbg:
        dbg_h = nc.dram_tensor("dbg_h", [P, nb * TBL], f32, kind="ExternalOutput")
        dbg_hf = nc.dram_tensor("dbg_hf", [P, TBL], f32, kind="ExternalOutput")
        dbg_g = nc.dram_tensor("dbg_g", [P, CHUNK_TILES * TBL], f32, kind="ExternalOutput")
        dbg_x1 = nc.dram_tensor("dbg_x1", [P, per], f32, kind="ExternalOutput")

    h_loc = [nc.dram_tensor(f"h_loc{l}", [per, TBL], f32) for l in range(nlayers)]
    h_full = [nc.dram_tensor(f"h_full{l}", [npad, TBL], f32, addr_space="Shared")
              for l in range(nlayers)]

    groups = [list(range(NCORES))]
    nchunks = Tpad // CHUNK_TILES

    with ExitStack() as ctx:
        tc = ctx.enter_context(tile.TileContext(nc))
        sb = ctx.enter_context(tc.tile_pool(name="sb", bufs=1))
        sb_g = ctx.enter_context(tc.tile_pool(name="sbg", bufs=3))
        sb_w = ctx.enter_context(tc.tile_pool(name="sbw", bufs=3))
        ps_h = ctx.enter_context(tc.tile_pool(name="psh", bufs=1, space="PSUM"))
        ps_agg = ctx.enter_context(tc.tile_pool(name="psagg", bufs=2, space="PSUM"))
        ps_st = ctx.enter_context(tc.tile_pool(name="psst", bufs=1, space="PSUM"))
        ps_xp = ctx.enter_context(tc.tile_pool(name="psxp", bufs=1, space="PSUM"))
        ps_fin = ctx.enter_context(tc.tile_pool(name="psfin", bufs=1, space="PSUM"))

        # ---- persistent SBUF state
        xT = sb.tile([P, per], f32)
        nc.sync.dma_start(out=xT[:], in_=xT_d[:])
        waug = sb.tile([P, nlayers, AUG], f32)
        nc.sync.dma_start(out=waug[:],
                          in_=waug_d[:].rearrange("l p a -> p l a"))
        wh = sb.tile([P, OUT], f32)
        nc.sync.dma_start(out=wh[:], in_=wh_d[:])
        iota = sb.tile([P, 4, P], f32)
        nc.sync.dma_start(out=iota[:].rearrange("p a b -> p (a b)"), in_=iota_d[:])
        srci = sb.tile([P, IW], mybir.dt.int32)
        nc.sync.dma_start(out=srci[:], in_=srcidx_d[:])
        dloc = sb.tile([P, Tpad, 1], f32)
        nc.sync.dma_start(out=dloc[:].rearrange("p t o -> p (t o)"), in_=dloc_d[:])
        batchf = sb.tile([P, nb, 1], f32)
        nc.sync.dma_start(out=batchf[:].rearrange("p b o -> p (b o)"), in_=batchf_d[:])
        ident = sb.tile([P, P], f32)
        make_identity(nc, ident[:])

        hsb = sb.tile([P, nb, TBL], f32)
        adsb = sb.tile([P, nb, H], f32)
        pooled_ps = ps_fin.tile([GRAPHS, HC], f32)

        for l in range(3):
            # ===== node phase: h_aug = x @ W_aug =====
            for b in range(nb):
                ps = ps_h.tile([P, AUG], f32)
                nc.tensor.matmul(ps[:], lhsT=xT[:, b * P:(b + 1) * P],
                                 rhs=waug[:, l, :], start=True, stop=True)
                nc.vector.tensor_copy(out=hsb[:, b, :], in_=ps[:, :TBL])
                nc.vector.tensor_copy(out=adsb[:, b, :], in_=ps[:, TBL:AUG])
            dh = nc.sync.dma_start(
                out=h_loc[l][:].rearrange("(b p) d -> p b d", p=P),
                in_=hsb[:])
            cch = nc.gpsimd.collective_compute(
                "AllGather", mybir.AluOpType.bypass, replica_groups=groups,
                ins=[h_loc[l][:]], outs=[h_full[l][:]])
            add_dep_helper(cch.ins, dh.ins, sync=True, reason="h write before ag")
            if dbg and l == 0:
                nc.sync.dma_start(out=dbg_h[:], in_=hsb[:].rearrange("p b d -> p (b d)"))
                hfs = sb.tile([P, TBL], f32)
                dhf = nc.sync.dma_start(out=hfs[:], in_=h_full[0][0:P, :].rearrange("(o p) d -> p (o d)", p=P))
                add_dep_helper(dhf.ins, cch.ins, sync=True, reason="dbg after ag")
                nc.sync.dma_start(out=dbg_hf[:], in_=hfs[:])

            # ===== edge phase =====
            agg_of_blk = {}
            for cidx in range(nchunks):
                t0 = cidx * CHUNK_TILES
                g = sb_g.tile([P, CHUNK_TILES, TBL], f32, tag="gath")
                for j in range(CHUNK_TILES):
                    gi = nc.gpsimd.indirect_dma_start(
                        out=g[:, j, :], out_offset=None, in_=h_full[l][:],
                        in_offset=IndirectOffsetOnAxis(
                            ap=srci[:, t0 + j:t0 + j + 1], axis=0))
                    add_dep_helper(gi.ins, cch.ins, sync=True, reason="gather after ag")
                if dbg and l == 0 and cidx == 0:
                    nc.sync.dma_start(out=dbg_g[:], in_=g[:].rearrange("p t d -> p (t d)"))

                for q in range(CHUNK_TILES // 4):
                    tq = t0 + 4 * q
                    sel4 = sb_w.tile([P, 4, P], f32, tag="sel")
                    nc.vector.tensor_tensor(
                        out=sel4[:],
                        in0=dloc[:, tq:tq + 4, :].to_broadcast([P, 4, P]),
                        in1=iota[:], op=mybir.AluOpType.is_equal)
                    lg4 = sb_w.tile([P, 4, H], f32, tag="lg")
                    for j in range(4):
                        t = tq + j
                        b = int(blk_of_tile[t])
                        stp = ps_st.tile([P, P], f32, tag="selT", name=f"st{l}_{t}")
                        nc.tensor.transpose(stp[:], sel4[:, j, :], ident[:])
                        sts = sb_w.tile([P, P], f32, tag="selTs", name=f"sts{l}_{t}")
                        nc.vector.tensor_copy(out=sts[:], in_=stp[:])
                        adp = ps_st.tile([P, H], f32, tag="ad4", name=f"ad{l}_{t}")
                        nc.tensor.matmul(adp[:], lhsT=sts[:], rhs=adsb[:, b, :],
                                         start=True, stop=True)
                        nc.vector.tensor_tensor(
                            out=lg4[:, j, :], in0=g[:, 4 * q + j, HC:TBL],
                            in1=adp[:], op=mybir.AluOpType.add)
                    lr4 = sb_w.tile([P, 4, H], f32, tag="lr")
                    nc.scalar.activation(lr4[:], lg4[:],
                                         mybir.ActivationFunctionType.Lrelu,
                                         alpha=NEG_SLOPE)
                    ex4 = sb_w.tile([P, 4, H], f32, tag="ex")
                    nc.scalar.activation(ex4[:], lr4[:],
                                         mybir.ActivationFunctionType.Exp)
                    msg4 = sb_w.tile([P, 4, TBL], f32, tag="msg")
                    for h in range(H):
                        nc.vector.tensor_tensor(
                            out=msg4[:, :, h * Ch:(h + 1) * Ch],
                            in0=g[:, 4 * q:4 * q + 4, h * Ch:(h + 1) * Ch],
                            in1=ex4[:, :, h:h + 1].to_broadcast([P, 4, Ch]),
                            op=mybir.AluOpType.mult)
                    nc.vector.tensor_copy(out=msg4[:, :, HC:TBL], in_=ex4[:])
                    for j in range(4):
                        t = tq + j
                        b = int(blk_of_tile[t])
                        if start_t[t]:
                            agg_of_blk[b] = ps_agg.tile([P, TBL], f32, tag="agg", name=f"agg{b}")
                        nc.tensor.matmul(agg_of_blk[b][:], lhsT=sel4[:, j, :],
                                         rhs=msg4[:, j, :],
                                         start=bool(start_t[t]),
                                         stop=bool(stop_t[t]))
                        if stop_t[t]:
                            agg = agg_of_blk.pop(b)
                            rec = sb_w.tile([P, H], f32, tag="rec")
                            nc.vector.reciprocal(rec[:], agg[:, HC:TBL])
                            xb = sb_w.tile([P, HC], f32, tag="xb")
                            for h in range(H):
                                nc.vector.tensor_tensor(
                                    out=xb[:, h * Ch:(h + 1) * Ch],
                                    in0=agg[:, h * Ch:(h + 1) * Ch],
                                    in1=rec[:, h:h + 1].to_broadcast([P, Ch]),
                                    op=mybir.AluOpType.mult)
                            nc.vector.tensor_scalar_max(xb[:], xb[:], 0.0)
                            if l < 2:
                                xps = ps_xp.tile([P, P], f32, tag="xps")
                                nc.tensor.transpose(xps[:], xb[:], ident[:])
                                nc.vector.tensor_copy(
                                    out=xT[:, b * P:(b + 1) * P], in_=xps[:])
                            else:
                                bsel = sb_w.tile([P, GRAPHS], f32, tag="bsel")
                                nc.vector.tensor_tensor(
                                    out=bsel[:],
                                    in0=batchf[:, b, :].to_broadcast([P, GRAPHS]),
                                    in1=iota[:, 0, :GRAPHS],
                                    op=mybir.AluOpType.is_equal)
                                nc.tensor.matmul(pooled_ps[:], lhsT=bsel[:],
                                                 rhs=xb[:], start=(b == 0),
                                                 stop=(b == nb - 1))

        if dbg:
            nc.sync.dma_start(out=dbg_x1[:], in_=xT[:])
        # ===== head =====
        pooled_sb = sb.tile([GRAPHS, HC], f32)
        nc.vector.tensor_copy(out=pooled_sb[:], in_=pooled_ps[:])
        pT_ps = ps_xp.tile([P, GRAPHS], f32, tag="xps")
        nc.tensor.transpose(pT_ps[:], pooled_sb[:], ident[:GRAPHS, :GRAPHS])
        pT_sb = sb.tile([P, GRAPHS], f32)
        nc.vector.tensor_copy(out=pT_sb[:], in_=pT_ps[:])
        log_ps = ps_xp.tile([GRAPHS, OUT], f32, tag="logps")
        nc.tensor.matmul(log_ps[:], lhsT=pT_sb[:], rhs=wh[:], start=True, stop=True)
        log_sb = sb.tile([GRAPHS, OUT], f32)
        nc.vector.tensor_copy(out=log_sb[:], in_=log_ps[:])
        nc.sync.dma_start(out=out_d[:], in_=log_sb[:])

    _fixup_wait_limits(nc)
    return nc


def prepare(x, Ws, a_srcs, a_dsts, biases, Wh, bh, edge_index, batch):
    n = x.shape[0]
    npad = int(math.ceil(n / (NCORES * P)) * NCORES * P)
    per = npad // NCORES
    nb = per // P

    x = np.asarray(x, np.float32)
    Ws = [np.asarray(w, np.float32) for w in Ws]
    a_srcs = [np.asarray(a, np.float32) for a in a_srcs]
    a_dsts = [np.asarray(a, np.float32) for a in a_dsts]
    Wh = np.asarray(Wh, np.float32)
    bh = np.asarray(bh, np.float32)
    edge_index = np.asarray(edge_index)
    batch = np.asarray(batch)
    for b in biases:
        assert np.allclose(np.asarray(b), 0.0), "nonzero GAT biases unsupported"

    # W_aug = [W | W@As | W@Ad]
    waugs = []
    for l in range(3):
        As = np.zeros((HC, H), np.float32)
        Ad = np.zeros((HC, H), np.float32)
        for h in range(H):
            As[h * Ch:(h + 1) * Ch, h] = a_srcs[l][h]
            Ad[h * Ch:(h + 1) * Ch, h] = a_dsts[l][h]
        W = Ws[l]
        waugs.append(np.concatenate([W, W @ As, W @ Ad], axis=1))
    waug = np.stack(waugs, 0)  # [3, 128, AUG]

    # edges + self loops (incl. pad nodes, so every row has >=1 edge)
    src_all = np.concatenate([edge_index[0].astype(np.int64),
                              np.arange(npad, dtype=np.int64)])
    dst_all = np.concatenate([edge_index[1].astype(np.int64),
                              np.arange(npad, dtype=np.int64)])
    blk_of_tile, start_t, stop_t, Tpad, srcs, adids, dlocs = _prep_edges(
        src_all, dst_all, per, nb)

    xpad = np.zeros((npad, HC), np.float32)
    xpad[:n] = x
    iota = np.tile(np.arange(P, dtype=np.float32)[None, :], (P, 4))

    batchf_full = np.full(npad, -1.0, np.float32)
    batchf_full[:n] = batch.astype(np.float32)

    nc = _build(npad, Tpad, blk_of_tile, start_t, stop_t)

    in_maps = []
    for c in range(NCORES):
        sl = slice(c * per, (c + 1) * per)
        in_maps.append({
            "xT": np.ascontiguousarray(xpad[sl].T),
            "waug": waug,
            "wh": Wh,
            "iota": iota,
            "srcidx": srcs[c],
            "dloc": dlocs[c],
            "batchf": np.ascontiguousarray(
                batchf_full[sl].reshape(nb, P).T),
            })
    return nc, in_maps


def run_gat(x, Ws, a_srcs, a_dsts, biases, Wh, bh, edge_index, batch):
    nc, in_maps = prepare(x, Ws, a_srcs, a_dsts, biases, Wh, bh,
                          edge_index, batch)
    res = run_bass_kernel_spmd(nc, in_maps, list(range(NCORES)))
    global LAST_EXEC_NS
    LAST_EXEC_NS = getattr(res, "exec_time_ns", None)
    logits = np.zeros((GRAPHS, OUT), np.float32)
    for c in range(NCORES):
        logits += res.results[c]["out"]
    return logits + bh


def kernel(**inputs):
    return np.asarray(run_gat(
        inputs["x"], inputs["Ws"], inputs["a_srcs"], inputs["a_dsts"],
        inputs["biases"], inputs["Wh"], inputs["bh"], inputs["edge_index"],
        inputs["batch"]), np.float32)
